# revision 15
# baseline (speedup 1.0000x reference)
"""Correlation module kernel for 8 TRN2 NeuronCores.

Reference computation (per batch element n, pure data-parallel over N):
    A_n = X_n @ U_n^T / sqrt(D)          # [L, O]
    W_n = sigmoid(A_n) - 0.5             # = 0.5 * tanh(A_n / 2)
    F_n = W_n @ U_n                      # [L, D]

Shapes: x [L=512, N=64, D=512] f32, upfold [O=512, N=64, D=512] f32.
Sharding: N axis across 8 cores (8 batch elements per core), no comms.

Device kernel (per core, per n):
    MM1:  psum_AT[o, l] = sum_d uT[d, o] * xT[d, l]      (fp16 in, f32 acc)
    ACT:  w[o, l] = tanh(psum_AT * 1/(2*sqrt(D)))        (-> fp16)
    MM2:  psum_F[l, d] = sum_o w[o, l] * (0.5*u)[o, d]   (fp16 in, f32 acc)
    DVE:  f[l, d] = psum_F                               (-> fp16)
    DMA out to y[l, n, d]; host upcasts to f32.

Host pre-arranges per-core inputs as fp16 in the exact layouts the PE
needs (d-major for MM1 operands, o-major for MM2's moving operand), so
the device does zero transposes and minimum HBM traffic (16.8 MB/core).

Schedule notes:
 - 8 dummy warm-up matmuls on scratch SBUF run while the first loads are
   in flight, so the PE's HAM clock gate reaches 8/8 before real work.
 - MM1 iterates d-major over 4 PSUM banks (one per o-block) so compute
   can start as soon as the first half of xT/uT lands; MM2 iterates
   o-major over 4 PSUM banks (one per l-block).
 - Loads are split across both HWDGE rings (sync: xT+u, scalar: uT) to
   halve the first-matmul latency; stores go on the scalar ring.
"""

import numpy as np

L, O, N, D = 512, 512, 64, 512
NCORES = 8
NLOC = N // NCORES  # 8 batch elements per core
P = 128  # SBUF partitions
DB = D // P  # 4 d-blocks
OB = O // P  # 4 o-blocks
LB = L // P  # 4 l-blocks
WARMUP_MMS = 12

_cache = {}


def _build_program():
    import concourse.bass as bass
    import concourse.mybir as mybir
    import concourse.tile as tile
    from concourse import bacc

    FP16 = mybir.dt.float16
    F32 = mybir.dt.float32
    Tanh = mybir.ActivationFunctionType.Tanh
    Copy = mybir.ActivationFunctionType.Copy

    nc = bacc.Bacc("TRN2", target_bir_lowering=False, debug=False)
    xt_d = nc.declare_dram_parameter("xt", [NLOC, D, L], FP16, isOutput=False)
    ut_d = nc.declare_dram_parameter("ut", [NLOC, D, O], FP16, isOutput=False)
    un_d = nc.declare_dram_parameter("un", [NLOC, O, D], FP16, isOutput=False)
    y_d = nc.declare_dram_parameter("y", [L, NLOC, D], FP16, isOutput=True)

    s2 = 1.0 / (2.0 * float(np.sqrt(D)))  # tanh half-argument scale

    with tile.TileContext(nc) as tc:
        with (
            tc.tile_pool(name="xt", bufs=NLOC) as xt_pool,
            tc.tile_pool(name="ut", bufs=NLOC) as ut_pool,
            tc.tile_pool(name="un", bufs=NLOC) as un_pool,
            tc.tile_pool(name="w", bufs=2) as w_pool,
            tc.tile_pool(name="fo", bufs=2) as f_pool,
            tc.tile_pool(name="scr", bufs=1) as scr_pool,
            tc.tile_pool(name="psa", bufs=1, space="PSUM") as psa_pool,
            tc.tile_pool(name="psf", bufs=1, space="PSUM") as psf_pool,
        ):
            # PE warm-up: dense dummy matmuls on (uninitialized) scratch
            # while the first loads are still in flight. Drives the HAM
            # activity window so real matmuls start at full clock.
            scr_t = scr_pool.tile([P, L], FP16, tag="scr")
            nc.gpsimd.memset(scr_t[:], 0.0)
            # trigger the ACT tanh table load now (1.3us) so the first real
            # tanh doesn't stall the MM1->MM2 pipeline on it
            scr2_t = scr_pool.tile([P, 1], FP16, tag="scr2")
            nc.scalar.activation(scr2_t[:], scr_t[:, 0:1], Tanh, scale=s2)
            ps_w = psa_pool.tile([P, L], F32, tag="psa0", name="ps_warm")
            for _ in range(WARMUP_MMS):
                nc.tensor.matmul(
                    ps_w[:], lhsT=scr_t[:, :P], rhs=scr_t[:], start=True, stop=True
                )

            for n in range(NLOC):
                # -- loads: xt halves + un on sync ring, ut halves on scalar --
                xt_t = xt_pool.tile([P, DB, L], FP16, tag="xt")
                ut_t = ut_pool.tile([P, DB, O], FP16, tag="ut")
                un_t = un_pool.tile([P, OB, D], FP16, tag="un")
                xt_ap = xt_d[n].rearrange("(b p) l -> p b l", p=P)
                ut_ap = ut_d[n].rearrange("(b p) o -> p b o", p=P)
                un_ap = un_d[n].rearrange("(b p) d -> p b d", p=P)
                nc.sync.dma_start(xt_t[:, 0:2, :], xt_ap[:, 0:2, :])
                nc.sync.dma_start(ut_t[:, 0:2, :], ut_ap[:, 0:2, :])
                nc.sync.dma_start(xt_t[:, 2:4, :], xt_ap[:, 2:4, :])
                nc.sync.dma_start(ut_t[:, 2:4, :], ut_ap[:, 2:4, :])
                nc.sync.dma_start(un_t[:], un_ap)

                # -- MM1 d-major over 4 PSUM banks: AT[o,l] += uT.T @ xT --
                ps_a = [
                    psa_pool.tile([P, L], F32, tag=f"psa{ob}", name=f"ps_a{ob}")
                    for ob in range(OB)
                ]
                # d-major for the first two d-blocks (compute starts as soon
                # as the first load halves land), then o-major so each
                # o-block's accumulation closes early and its tanh overlaps
                # the remaining matmuls.
                mm1_order = [(db, ob) for db in range(2) for ob in range(OB)]
                mm1_order += [(db, ob) for ob in range(OB) for db in range(2, DB)]
                for db, ob in mm1_order:
                    nc.tensor.matmul(
                        ps_a[ob][:],
                        lhsT=ut_t[:, db, bass.ts(ob, P)],
                        rhs=xt_t[:, db, :],
                        start=(db == 0),
                        stop=(db == DB - 1),
                    )
                # -- sigmoid-center: w = tanh(AT * s2)  (fp16) --
                w_t = w_pool.tile([P, OB, L], FP16, tag="w")
                for ob in range(OB):
                    nc.scalar.activation(w_t[:, ob, :], ps_a[ob][:], Tanh, scale=s2)

                # -- MM2 o-major over 4 PSUM banks: F[l,d] += w.T @ un --
                ps_f = [
                    psf_pool.tile([P, D], F32, tag=f"psf{lb}", name=f"ps_f{lb}")
                    for lb in range(LB)
                ]
                # o-major lets MM2 start with just w[0] ready; for the last
                # batch element close each l-block early (lb-major) so the
                # casts/stores pipeline during the final matmuls.
                last = n == NLOC - 1
                if last:
                    mm2_order = [(ob, lb) for lb in range(LB) for ob in range(OB)]
                else:
                    mm2_order = [(ob, lb) for ob in range(OB) for lb in range(LB)]
                for ob, lb in mm2_order:
                    nc.tensor.matmul(
                        ps_f[lb][:],
                        lhsT=w_t[:, ob, bass.ts(lb, P)],
                        rhs=un_t[:, ob, :],
                        start=(ob == 0),
                        stop=(ob == OB - 1),
                    )
                # -- PSUM -> SBUF casts split across DVE and ACT; store each
                # quarter via SWDGE as soon as its cast lands (gpsimd is
                # otherwise idle; keeps DMA issue off the ACT/Sync rings) --
                f_t = f_pool.tile([P, LB, D], FP16, tag="f")
                y_ap = y_d[:, n, :].rearrange("(b p) d -> p b d", p=P)
                for lb in range(LB):
                    if lb % 2 == 0:
                        nc.vector.tensor_copy(f_t[:, lb, :], ps_f[lb][:])
                    else:
                        nc.scalar.activation(f_t[:, lb, :], ps_f[lb][:], Copy)
                    if last and lb % 2 == 1:
                        nc.sync.dma_start(y_ap[:, lb, :], f_t[:, lb, :])
                    else:
                        nc.gpsimd.dma_start(y_ap[:, lb, :], f_t[:, lb, :])
    nc.compile()
    return nc


def _prepare_in_maps(x, u):
    f16 = np.float16
    in_maps = []
    for c in range(NCORES):
        ns = slice(c * NLOC, (c + 1) * NLOC)
        xs = x[:, ns, :]  # [L, NLOC, D]
        us = u[:, ns, :]  # [O, NLOC, D]
        in_maps.append(
            {
                # X^T per n: [NLOC, D, L]
                "xt": np.ascontiguousarray(xs.transpose(1, 2, 0)).astype(f16),
                # U^T per n: [NLOC, D, O]
                "ut": np.ascontiguousarray(us.transpose(1, 2, 0)).astype(f16),
                # U natural per n, pre-scaled by 0.5 (folds sigmoid's -0.5
                # via sigmoid(a)-0.5 = 0.5*tanh(a/2)): [NLOC, O, D]
                "un": (0.5 * us.transpose(1, 0, 2)).astype(f16),
            }
        )
    return in_maps


def _run(inputs, trace=False, **spmd_kwargs):
    from concourse.bass_utils import run_bass_kernel_spmd

    x = np.asarray(inputs["x"], dtype=np.float32)
    u = np.asarray(inputs["upfold"], dtype=np.float32)
    assert x.shape == (L, N, D) and u.shape == (O, N, D)

    if "nc" not in _cache:
        _cache["nc"] = _build_program()
    nc = _cache["nc"]

    in_maps = _prepare_in_maps(x, u)
    res = run_bass_kernel_spmd(
        nc, in_maps, core_ids=list(range(NCORES)), trace=trace, **spmd_kwargs
    )
    out = np.concatenate([r["y"] for r in res.results], axis=1)  # [L, N, D]
    return np.ascontiguousarray(out.astype(np.float32)), res


def kernel(**inputs) -> np.ndarray:
    out, _ = _run(inputs, trace=False)
    return out


# revision 17
# speedup vs baseline: 1.0001x; 1.0001x over previous
"""Correlation module kernel for 8 TRN2 NeuronCores.

Reference computation (per batch element n, pure data-parallel over N):
    A_n = X_n @ U_n^T / sqrt(D)          # [L, O]
    W_n = sigmoid(A_n) - 0.5             # = 0.5 * tanh(A_n / 2)
    F_n = W_n @ U_n                      # [L, D]

Shapes: x [L=512, N=64, D=512] f32, upfold [O=512, N=64, D=512] f32.
Sharding: N axis across 8 cores (8 batch elements per core), no comms.

Device kernel (per core, per n):
    MM1:  psum_AT[o, l] = sum_d uT[d, o] * xT[d, l]      (fp16 in, f32 acc)
    ACT:  w[o, l] = tanh(psum_AT * 1/(2*sqrt(D)))        (-> fp16)
    MM2:  psum_F[l, d] = sum_o w[o, l] * (0.5*u)[o, d]   (fp16 in, f32 acc)
    DVE:  f[l, d] = psum_F                               (-> fp16)
    DMA out to y[l, n, d]; host upcasts to f32.

Host pre-arranges per-core inputs as fp16 in the exact layouts the PE
needs (d-major for MM1 operands, o-major for MM2's moving operand), so
the device does zero transposes and minimum HBM traffic (16.8 MB/core).

Schedule notes:
 - 8 dummy warm-up matmuls on scratch SBUF run while the first loads are
   in flight, so the PE's HAM clock gate reaches 8/8 before real work.
 - MM1 iterates d-major over 4 PSUM banks (one per o-block) so compute
   can start as soon as the first half of xT/uT lands; MM2 iterates
   o-major over 4 PSUM banks (one per l-block).
 - Loads are split across both HWDGE rings (sync: xT+u, scalar: uT) to
   halve the first-matmul latency; stores go on the scalar ring.
"""

import numpy as np

L, O, N, D = 512, 512, 64, 512
NCORES = 8
NLOC = N // NCORES  # 8 batch elements per core
P = 128  # SBUF partitions
DB = D // P  # 4 d-blocks
OB = O // P  # 4 o-blocks
LB = L // P  # 4 l-blocks
WARMUP_MMS = 12

_cache = {}


def _build_program():
    import concourse.bass as bass
    import concourse.mybir as mybir
    import concourse.tile as tile
    from concourse import bacc

    FP16 = mybir.dt.float16
    F32 = mybir.dt.float32
    Tanh = mybir.ActivationFunctionType.Tanh
    Copy = mybir.ActivationFunctionType.Copy

    nc = bacc.Bacc("TRN2", target_bir_lowering=False, debug=False)
    xt_d = nc.declare_dram_parameter("xt", [NLOC, D, L], FP16, isOutput=False)
    ut_d = nc.declare_dram_parameter("ut", [NLOC, D, O], FP16, isOutput=False)
    un_d = nc.declare_dram_parameter("un", [NLOC, O, D], FP16, isOutput=False)
    y_d = nc.declare_dram_parameter("y", [L, NLOC, D], FP16, isOutput=True)

    s2 = 1.0 / (2.0 * float(np.sqrt(D)))  # tanh half-argument scale

    with tile.TileContext(nc) as tc:
        with (
            tc.tile_pool(name="xt", bufs=NLOC) as xt_pool,
            tc.tile_pool(name="ut", bufs=NLOC) as ut_pool,
            tc.tile_pool(name="un", bufs=NLOC) as un_pool,
            tc.tile_pool(name="w", bufs=2) as w_pool,
            tc.tile_pool(name="fo", bufs=2) as f_pool,
            tc.tile_pool(name="scr", bufs=1) as scr_pool,
            tc.tile_pool(name="psa", bufs=1, space="PSUM") as psa_pool,
            tc.tile_pool(name="psf", bufs=1, space="PSUM") as psf_pool,
        ):
            # PE warm-up: dense dummy matmuls on (uninitialized) scratch
            # while the first loads are still in flight. Drives the HAM
            # activity window so real matmuls start at full clock.
            scr_t = scr_pool.tile([P, L], FP16, tag="scr")
            nc.gpsimd.memset(scr_t[:], 0.0)
            # trigger the ACT tanh table load now (1.3us) so the first real
            # tanh doesn't stall the MM1->MM2 pipeline on it
            scr2_t = scr_pool.tile([P, 1], FP16, tag="scr2")
            nc.scalar.activation(scr2_t[:], scr_t[:, 0:1], Tanh, scale=s2)
            ps_w = psa_pool.tile([P, L], F32, tag="psa0", name="ps_warm")
            for _ in range(WARMUP_MMS):
                nc.tensor.matmul(
                    ps_w[:], lhsT=scr_t[:, :P], rhs=scr_t[:], start=True, stop=True
                )

            for n in range(NLOC):
                # -- loads: xt halves + un on sync ring, ut halves on scalar --
                xt_t = xt_pool.tile([P, DB, L], FP16, tag="xt")
                ut_t = ut_pool.tile([P, DB, O], FP16, tag="ut")
                un_t = un_pool.tile([P, OB, D], FP16, tag="un")
                xt_ap = xt_d[n].rearrange("(b p) l -> p b l", p=P)
                ut_ap = ut_d[n].rearrange("(b p) o -> p b o", p=P)
                un_ap = un_d[n].rearrange("(b p) d -> p b d", p=P)
                nc.sync.dma_start(xt_t[:, 0:2, :], xt_ap[:, 0:2, :])
                nc.sync.dma_start(ut_t[:, 0:2, :], ut_ap[:, 0:2, :])
                nc.sync.dma_start(xt_t[:, 2:4, :], xt_ap[:, 2:4, :])
                nc.sync.dma_start(ut_t[:, 2:4, :], ut_ap[:, 2:4, :])
                nc.sync.dma_start(un_t[:], un_ap)

                # -- MM1 d-major over 4 PSUM banks: AT[o,l] += uT.T @ xT --
                ps_a = [
                    psa_pool.tile([P, L], F32, tag=f"psa{ob}", name=f"ps_a{ob}")
                    for ob in range(OB)
                ]
                # d-major for the first two d-blocks (compute starts as soon
                # as the first load halves land), then o-major so each
                # o-block's accumulation closes early and its tanh overlaps
                # the remaining matmuls.
                mm1_order = [(db, ob) for db in range(2) for ob in range(OB)]
                mm1_order += [(db, ob) for ob in range(OB) for db in range(2, DB)]
                for db, ob in mm1_order:
                    nc.tensor.matmul(
                        ps_a[ob][:],
                        lhsT=ut_t[:, db, bass.ts(ob, P)],
                        rhs=xt_t[:, db, :],
                        start=(db == 0),
                        stop=(db == DB - 1),
                    )
                # -- sigmoid-center: w = tanh(AT * s2)  (fp16) --
                w_t = w_pool.tile([P, OB, L], FP16, tag="w")
                for ob in range(OB):
                    nc.scalar.activation(w_t[:, ob, :], ps_a[ob][:], Tanh, scale=s2)

                # -- MM2 o-major over 4 PSUM banks: F[l,d] += w.T @ un --
                ps_f = [
                    psf_pool.tile([P, D], F32, tag=f"psf{lb}", name=f"ps_f{lb}")
                    for lb in range(LB)
                ]
                # o-major lets MM2 start with just w[0] ready; for the last
                # batch element close each l-block early (lb-major) so the
                # casts/stores pipeline during the final matmuls.
                last = n == NLOC - 1
                if last:
                    mm2_order = [(ob, lb) for lb in range(LB) for ob in range(OB)]
                else:
                    mm2_order = [(ob, lb) for ob in range(OB) for lb in range(LB)]
                for ob, lb in mm2_order:
                    nc.tensor.matmul(
                        ps_f[lb][:],
                        lhsT=w_t[:, ob, bass.ts(lb, P)],
                        rhs=un_t[:, ob, :],
                        start=(ob == 0),
                        stop=(ob == OB - 1),
                    )
                # -- PSUM -> SBUF casts split across DVE and ACT; store each
                # quarter via SWDGE as soon as its cast lands (gpsimd is
                # otherwise idle; keeps DMA issue off the ACT/Sync rings) --
                f_t = f_pool.tile([P, LB, D], FP16, tag="f")
                y_ap = y_d[:, n, :].rearrange("(b p) d -> p b d", p=P)
                for lb in range(LB):
                    if lb % 2 == 0:
                        nc.vector.tensor_copy(f_t[:, lb, :], ps_f[lb][:])
                    else:
                        nc.scalar.activation(f_t[:, lb, :], ps_f[lb][:], Copy)
                    if last and lb % 2 == 1:
                        nc.sync.dma_start(y_ap[:, lb, :], f_t[:, lb, :])
                    else:
                        nc.gpsimd.dma_start(y_ap[:, lb, :], f_t[:, lb, :])
    nc.compile()
    return nc


def _prepare_in_maps(x, u):
    f16 = np.float16
    in_maps = []
    for c in range(NCORES):
        ns = slice(c * NLOC, (c + 1) * NLOC)
        xs = x[:, ns, :]  # [L, NLOC, D]
        us = u[:, ns, :]  # [O, NLOC, D]
        in_maps.append(
            {
                # X^T per n: [NLOC, D, L]
                "xt": np.ascontiguousarray(xs.transpose(1, 2, 0)).astype(f16),
                # U^T per n: [NLOC, D, O]
                "ut": np.ascontiguousarray(us.transpose(1, 2, 0)).astype(f16),
                # U natural per n, pre-scaled by 0.5 (folds sigmoid's -0.5
                # via sigmoid(a)-0.5 = 0.5*tanh(a/2)): [NLOC, O, D]
                "un": (0.5 * us.transpose(1, 0, 2)).astype(f16),
            }
        )
    return in_maps


def _run(inputs, trace=False, **spmd_kwargs):
    from concourse.bass_utils import run_bass_kernel_spmd

    x = np.asarray(inputs["x"], dtype=np.float32)
    u = np.asarray(inputs["upfold"], dtype=np.float32)
    assert x.shape == (L, N, D) and u.shape == (O, N, D)

    if "nc" not in _cache:
        _cache["nc"] = _build_program()
    nc = _cache["nc"]

    in_maps = _prepare_in_maps(x, u)
    res = run_bass_kernel_spmd(
        nc, in_maps, core_ids=list(range(NCORES)), trace=trace, **spmd_kwargs
    )
    out = np.concatenate([r["y"] for r in res.results], axis=1)  # [L, N, D]
    return np.ascontiguousarray(out.astype(np.float32)), res


def kernel(**inputs) -> np.ndarray:
    out, _ = _run(inputs, trace=False)
    return out


# revision 18
# speedup vs baseline: 1.0113x; 1.0112x over previous
"""Correlation module kernel for 8 TRN2 NeuronCores.

Reference computation (per batch element n, pure data-parallel over N):
    A_n = X_n @ U_n^T / sqrt(D)          # [L, O]
    W_n = sigmoid(A_n) - 0.5             # = 0.5 * tanh(A_n / 2)
    F_n = W_n @ U_n                      # [L, D]

Shapes: x [L=512, N=64, D=512] f32, upfold [O=512, N=64, D=512] f32.
Sharding: N axis across 8 cores (8 batch elements per core), no comms.

Device kernel (per core, per n):
    MM1:  psum_AT[o, l] = sum_d uT[d, o] * xT[d, l]      (fp16 in, f32 acc)
    ACT:  w[o, l] = tanh(psum_AT * 1/(2*sqrt(D)))        (-> fp16)
    MM2:  psum_F[l, d] = sum_o w[o, l] * (0.5*u)[o, d]   (fp16 in, f32 acc)
    DVE:  f[l, d] = psum_F                               (-> fp16)
    DMA out to y[l, n, d]; host upcasts to f32.

Host pre-arranges per-core inputs as fp16 in the exact layouts the PE
needs (d-major for MM1 operands, o-major for MM2's moving operand), so
the device does zero transposes and minimum HBM traffic (16.8 MB/core).

Schedule notes:
 - 12 dummy warm-up matmuls on scratch SBUF run while the first loads
   are in flight, so the PE's HAM clock gate reaches 8/8 before real
   work; a dummy tanh pre-triggers the ACT table load (1.3us).
 - MM1 starts d-major over 4 PSUM banks (compute starts as soon as the
   first half of xT/uT lands) then closes each o-block early so its
   tanh overlaps the remaining matmuls; MM2 runs o-major over 4 more
   PSUM banks (starts with just w[0] ready); the last batch element
   instead closes l-blocks early so casts/stores drain during the
   final matmuls.
 - All loads ride the sync HWDGE ring (keeping DMA descriptor
   generation off the ACT ring, which runs the tanhs); stores ride the
   otherwise-idle gpsimd SWDGE ring, except the last element's odd
   quarters which use the by-then-idle sync ring.
"""

import numpy as np

L, O, N, D = 512, 512, 64, 512
NCORES = 8
NLOC = N // NCORES  # 8 batch elements per core
P = 128  # SBUF partitions
DB = D // P  # 4 d-blocks
OB = O // P  # 4 o-blocks
LB = L // P  # 4 l-blocks
WARMUP_MMS = 12

_cache = {}


def _build_program():
    import concourse.bass as bass
    import concourse.mybir as mybir
    import concourse.tile as tile
    from concourse import bacc

    FP16 = mybir.dt.float16
    F32 = mybir.dt.float32
    Tanh = mybir.ActivationFunctionType.Tanh
    Copy = mybir.ActivationFunctionType.Copy

    nc = bacc.Bacc("TRN2", target_bir_lowering=False, debug=False)
    xt_d = nc.declare_dram_parameter("xt", [NLOC, D, L], FP16, isOutput=False)
    ut_d = nc.declare_dram_parameter("ut", [NLOC, D, O], FP16, isOutput=False)
    un_d = nc.declare_dram_parameter("un", [NLOC, O, D], FP16, isOutput=False)
    y_d = nc.declare_dram_parameter("y", [L, NLOC, D], FP16, isOutput=True)

    s2 = 1.0 / (2.0 * float(np.sqrt(D)))  # tanh half-argument scale

    with tile.TileContext(nc) as tc:
        with (
            tc.tile_pool(name="xt", bufs=NLOC) as xt_pool,
            tc.tile_pool(name="ut", bufs=NLOC) as ut_pool,
            tc.tile_pool(name="un", bufs=NLOC) as un_pool,
            tc.tile_pool(name="w", bufs=2) as w_pool,
            tc.tile_pool(name="fo", bufs=2) as f_pool,
            tc.tile_pool(name="scr", bufs=1) as scr_pool,
            tc.tile_pool(name="psa", bufs=1, space="PSUM") as psa_pool,
            tc.tile_pool(name="psf", bufs=1, space="PSUM") as psf_pool,
        ):
            # PE warm-up: dense dummy matmuls on (uninitialized) scratch
            # while the first loads are still in flight. Drives the HAM
            # activity window so real matmuls start at full clock.
            scr_t = scr_pool.tile([P, L], FP16, tag="scr")
            nc.gpsimd.memset(scr_t[:], 0.0)
            # trigger the ACT tanh table load now (1.3us) so the first real
            # tanh doesn't stall the MM1->MM2 pipeline on it
            scr2_t = scr_pool.tile([P, 1], FP16, tag="scr2")
            nc.scalar.activation(scr2_t[:], scr_t[:, 0:1], Tanh, scale=s2)
            ps_w = psa_pool.tile([P, L], F32, tag="psa0", name="ps_warm")
            for _ in range(WARMUP_MMS):
                nc.tensor.matmul(
                    ps_w[:], lhsT=scr_t[:, :P], rhs=scr_t[:], start=True, stop=True
                )

            for n in range(NLOC):
                # -- loads: xt halves + un on sync ring, ut halves on scalar --
                xt_t = xt_pool.tile([P, DB, L], FP16, tag="xt")
                ut_t = ut_pool.tile([P, DB, O], FP16, tag="ut")
                un_t = un_pool.tile([P, OB, D], FP16, tag="un")
                xt_ap = xt_d[n].rearrange("(b p) l -> p b l", p=P)
                ut_ap = ut_d[n].rearrange("(b p) o -> p b o", p=P)
                un_ap = un_d[n].rearrange("(b p) d -> p b d", p=P)
                nc.sync.dma_start(xt_t[:, 0:2, :], xt_ap[:, 0:2, :])
                nc.sync.dma_start(ut_t[:, 0:2, :], ut_ap[:, 0:2, :])
                nc.sync.dma_start(xt_t[:, 2:4, :], xt_ap[:, 2:4, :])
                nc.sync.dma_start(ut_t[:, 2:4, :], ut_ap[:, 2:4, :])
                nc.sync.dma_start(un_t[:], un_ap)

                # -- MM1 d-major over 4 PSUM banks: AT[o,l] += uT.T @ xT --
                ps_a = [
                    psa_pool.tile([P, L], F32, tag=f"psa{ob}", name=f"ps_a{ob}")
                    for ob in range(OB)
                ]
                # d-major for the first two d-blocks (compute starts as soon
                # as the first load halves land), then o-major so each
                # o-block's accumulation closes early and its tanh overlaps
                # the remaining matmuls.
                mm1_order = [(db, ob) for db in range(2) for ob in range(OB)]
                mm1_order += [(db, ob) for ob in range(OB) for db in range(2, DB)]
                for db, ob in mm1_order:
                    nc.tensor.matmul(
                        ps_a[ob][:],
                        lhsT=ut_t[:, db, bass.ts(ob, P)],
                        rhs=xt_t[:, db, :],
                        start=(db == 0),
                        stop=(db == DB - 1),
                    )
                # -- sigmoid-center: w = tanh(AT * s2)  (fp16) --
                w_t = w_pool.tile([P, OB, L], FP16, tag="w")
                for ob in range(OB):
                    nc.scalar.activation(w_t[:, ob, :], ps_a[ob][:], Tanh, scale=s2)

                # -- MM2 o-major over 4 PSUM banks: F[l,d] += w.T @ un --
                ps_f = [
                    psf_pool.tile([P, D], F32, tag=f"psf{lb}", name=f"ps_f{lb}")
                    for lb in range(LB)
                ]
                # o-major lets MM2 start with just w[0] ready; for the last
                # batch element close each l-block early (lb-major) so the
                # casts/stores pipeline during the final matmuls.
                last = n == NLOC - 1
                if last:
                    mm2_order = [(ob, lb) for lb in range(LB) for ob in range(OB)]
                else:
                    mm2_order = [(ob, lb) for ob in range(OB) for lb in range(LB)]
                for ob, lb in mm2_order:
                    nc.tensor.matmul(
                        ps_f[lb][:],
                        lhsT=w_t[:, ob, bass.ts(lb, P)],
                        rhs=un_t[:, ob, :],
                        start=(ob == 0),
                        stop=(ob == OB - 1),
                    )
                # -- PSUM -> SBUF casts split across DVE and ACT; store each
                # quarter via SWDGE as soon as its cast lands (gpsimd is
                # otherwise idle; keeps DMA issue off the ACT/Sync rings) --
                f_t = f_pool.tile([P, LB, D], FP16, tag="f")
                y_ap = y_d[:, n, :].rearrange("(b p) d -> p b d", p=P)
                for lb in range(LB):
                    if lb % 2 == 0:
                        nc.vector.tensor_copy(f_t[:, lb, :], ps_f[lb][:])
                    else:
                        nc.scalar.activation(f_t[:, lb, :], ps_f[lb][:], Copy)
                    if last and lb % 2 == 1:
                        nc.sync.dma_start(y_ap[:, lb, :], f_t[:, lb, :])
                    else:
                        nc.gpsimd.dma_start(y_ap[:, lb, :], f_t[:, lb, :])
    nc.compile()
    return nc


def _prepare_in_maps(x, u):
    f16 = np.float16
    in_maps = []
    for c in range(NCORES):
        ns = slice(c * NLOC, (c + 1) * NLOC)
        xs = x[:, ns, :]  # [L, NLOC, D]
        us = u[:, ns, :]  # [O, NLOC, D]
        in_maps.append(
            {
                # X^T per n: [NLOC, D, L]
                "xt": np.ascontiguousarray(xs.transpose(1, 2, 0)).astype(f16),
                # U^T per n: [NLOC, D, O]
                "ut": np.ascontiguousarray(us.transpose(1, 2, 0)).astype(f16),
                # U natural per n, pre-scaled by 0.5 (folds sigmoid's -0.5
                # via sigmoid(a)-0.5 = 0.5*tanh(a/2)): [NLOC, O, D]
                "un": (0.5 * us.transpose(1, 0, 2)).astype(f16),
            }
        )
    return in_maps


def _run(inputs, trace=False, **spmd_kwargs):
    from concourse.bass_utils import run_bass_kernel_spmd

    x = np.asarray(inputs["x"], dtype=np.float32)
    u = np.asarray(inputs["upfold"], dtype=np.float32)
    assert x.shape == (L, N, D) and u.shape == (O, N, D)

    if "nc" not in _cache:
        _cache["nc"] = _build_program()
    nc = _cache["nc"]

    in_maps = _prepare_in_maps(x, u)
    res = run_bass_kernel_spmd(
        nc, in_maps, core_ids=list(range(NCORES)), trace=trace, **spmd_kwargs
    )
    out = np.concatenate([r["y"] for r in res.results], axis=1)  # [L, N, D]
    return np.ascontiguousarray(out.astype(np.float32)), res


def kernel(**inputs) -> np.ndarray:
    out, _ = _run(inputs, trace=False)
    return out
